# revision 6
# baseline (speedup 1.0000x reference)
"""BailingMoe (T=1024, H=1024, E=16, K=4, I=1408, IS=2816) on 8 TRN2 cores.

Strategy: expert-parallel, 2 experts per core. The router (x @ w_gate,
softmax, top-4, renorm — 0.02% of FLOPs) runs on host as part of input
sharding: tokens are gathered per expert (capacity C = max expert load,
rounded up to 64) and shipped pre-transposed. Each core computes its two
experts' MLPs on their gathered tokens (bf16 matmuls, f32 PSUM
accumulation), scales rows by the renormalized combine weight, and also
computes a tensor-parallel slice (IS/8 = 352, zero-padded to 384) of the
shared expert over all tokens. Host unshards: scatter-add the weighted
expert outputs and sum the 8 shared-expert partials.
"""

import functools

import numpy as np
import ml_dtypes

T = 1024
H = 1024
E = 16
K = 4
I = 1408
IS = 2816
ISP = 384          # padded per-core shared-expert slice (2816/8 = 352 -> 384)
TI = I // 128      # 11 intermediate tiles per routed expert
N_CORES = 8

BF16 = ml_dtypes.bfloat16


def _build_nc(C: int):
    import concourse.bass as bass  # noqa: F401  (bacc needs bass loaded)
    import concourse.mybir as mybir
    import concourse.tile as tile
    from concourse import bacc

    BF = mybir.dt.bfloat16
    F32 = mybir.dt.float32

    nc = bacc.Bacc(None, target_bir_lowering=False, debug=False)

    xT_ext = nc.declare_dram_parameter("xT", [H, T], BF, isOutput=False)
    xe_ext = nc.declare_dram_parameter("xe", [H, 2 * C], BF, isOutput=False)
    wtok_ext = nc.declare_dram_parameter("wtok", [2 * C, 1], F32, isOutput=False)
    wgu_ext = nc.declare_dram_parameter("w_gu", [2, H, 2 * I], BF, isOutput=False)
    wdn_ext = nc.declare_dram_parameter("w_dn", [2, I, H], BF, isOutput=False)
    wsgu_ext = nc.declare_dram_parameter("w_sgu", [H, 2 * ISP], BF, isOutput=False)
    wsd_ext = nc.declare_dram_parameter("w_sd", [ISP, H], BF, isOutput=False)
    out_ext = nc.declare_dram_parameter("out", [2 * C + T, H], F32, isOutput=True)

    # ragged token tiles per expert (C is a multiple of 64)
    tt_sizes = [128] * (C // 128) + ([C % 128] if C % 128 else [])

    with tile.TileContext(nc) as tc:
        with (
            tc.tile_pool(name="xpool", bufs=1) as xpool,
            tc.tile_pool(name="wg_pool", bufs=4) as wg_pool,
            tc.tile_pool(name="wu_pool", bufs=4) as wu_pool,
            tc.tile_pool(name="wdn_pool", bufs=2) as wdn_pool,
            tc.tile_pool(name="act_pool", bufs=2) as act_pool,
            tc.tile_pool(name="spool", bufs=1) as spool,
            tc.tile_pool(name="tmp_pool", bufs=3) as tmp_pool,
            tc.tile_pool(name="y_pool", bufs=4) as y_pool,
            tc.tile_pool(name="wt_pool", bufs=1) as wt_pool,
            tc.tile_pool(name="pg", bufs=2, space="PSUM") as pg,
            tc.tile_pool(name="pd", bufs=3, space="PSUM") as pd,
        ):
            # ---- persistent inputs ----
            xsb = xpool.tile([128, 8, T], BF, tag="xsb")
            nc.sync.dma_start(xsb[:], xT_ext[:].rearrange("(o p) t -> p o t", p=128))
            xesb = xpool.tile([128, 8, 2 * C], BF, tag="xesb")
            nc.sync.dma_start(xesb[:], xe_ext[:].rearrange("(o p) t -> p o t", p=128))

            # combine weights, one [tw, 1] strip per (expert, token tile)
            wt_tiles = {}
            for e in range(2):
                t0 = 0
                for tt, tw in enumerate(tt_sizes):
                    w = wt_pool.tile([128, 1], F32, tag=f"wt_{e}_{tt}")
                    nc.sync.dma_start(
                        w[:tw], wtok_ext[e * C + t0 : e * C + t0 + tw, :]
                    )
                    wt_tiles[(e, tt)] = w
                    t0 += tw

            acte = [None, None]

            def gate_up(e):
                """gu^T tiles [128 of 2I, C] -> silu(g)*u -> acte[e][128, TI, C]"""
                cb = e * C
                a = act_pool.tile([128, TI, C], BF, tag="acte")
                acte[e] = a
                for i in range(TI):
                    wg = wg_pool.tile([128, 8, 128], BF, tag="wg")
                    nc.sync.dma_start(
                        wg[:],
                        wgu_ext[e, :, i * 128 : (i + 1) * 128].rearrange(
                            "(o p) c -> p o c", p=128
                        ),
                    )
                    wu = wu_pool.tile([128, 8, 128], BF, tag="wu")
                    nc.sync.dma_start(
                        wu[:],
                        wgu_ext[e, :, I + i * 128 : I + (i + 1) * 128].rearrange(
                            "(o p) c -> p o c", p=128
                        ),
                    )
                    psg = pg.tile([128, 512], F32, tag="psg", name="psg")[:, :C]
                    psu = pg.tile([128, 512], F32, tag="psu", name="psu")[:, :C]
                    for h in range(8):
                        rhs = xesb[:, h, cb : cb + C]
                        nc.tensor.matmul(
                            psg, wg[:, h, :], rhs, start=(h == 0), stop=(h == 7)
                        )
                        nc.tensor.matmul(
                            psu, wu[:, h, :], rhs, start=(h == 0), stop=(h == 7)
                        )
                    tmp = tmp_pool.tile([128, 512], F32, tag="tmp", name="tmp")[:, :C]
                    nc.scalar.activation(
                        tmp, psg, mybir.ActivationFunctionType.Silu
                    )
                    nc.vector.tensor_mul(a[:, i, :], tmp, psu)

            def load_wdn(e):
                w = wdn_pool.tile([128, TI, H], BF, tag="wdn")
                for io in range(TI):
                    nc.sync.dma_start(
                        w[:, io, :], wdn_ext[e, io * 128 : (io + 1) * 128, :]
                    )
                return w

            def down(e, wdn):
                a = acte[e]
                t0 = 0
                for tt, tw in enumerate(tt_sizes):
                    for hc in range(2):
                        ps = pd.tile([128, 512], F32, tag="pd", name="pd")[:tw]
                        for io in range(TI):
                            nc.tensor.matmul(
                                ps,
                                a[:, io, t0 : t0 + tw],
                                wdn[:, io, hc * 512 : (hc + 1) * 512],
                                start=(io == 0),
                                stop=(io == TI - 1),
                            )
                        y = y_pool.tile([128, 512], F32, tag="y", name="y")[:tw]
                        nc.vector.tensor_scalar_mul(
                            y, ps, wt_tiles[(e, tt)][:tw]
                        )
                        nc.sync.dma_start(
                            out_ext[
                                e * C + t0 : e * C + t0 + tw,
                                hc * 512 : (hc + 1) * 512,
                            ],
                            y,
                        )
                    t0 += tw

            def shared_gate_up():
                acts = spool.tile([128, ISP // 128, T], BF, tag="acts")
                for j in range(ISP // 128):
                    wsg = wg_pool.tile([128, 8, 128], BF, tag="wsg")
                    nc.sync.dma_start(
                        wsg[:],
                        wsgu_ext[:, j * 128 : (j + 1) * 128].rearrange(
                            "(o p) c -> p o c", p=128
                        ),
                    )
                    wsu = wu_pool.tile([128, 8, 128], BF, tag="wsu")
                    nc.sync.dma_start(
                        wsu[:],
                        wsgu_ext[:, ISP + j * 128 : ISP + (j + 1) * 128].rearrange(
                            "(o p) c -> p o c", p=128
                        ),
                    )
                    for tch in range(2):
                        psg = pg.tile([128, 512], F32, tag="psg")
                        psu = pg.tile([128, 512], F32, tag="psu")
                        rhs = xsb[:, :, tch * 512 : (tch + 1) * 512]
                        for h in range(8):
                            nc.tensor.matmul(
                                psg,
                                wsg[:, h, :],
                                rhs[:, h, :],
                                start=(h == 0),
                                stop=(h == 7),
                            )
                            nc.tensor.matmul(
                                psu,
                                wsu[:, h, :],
                                rhs[:, h, :],
                                start=(h == 0),
                                stop=(h == 7),
                            )
                        tmp = tmp_pool.tile([128, 512], F32, tag="tmp")
                        nc.scalar.activation(
                            tmp, psg, mybir.ActivationFunctionType.Silu
                        )
                        nc.vector.tensor_mul(
                            acts[:, j, tch * 512 : (tch + 1) * 512], tmp, psu
                        )
                return acts

            def shared_down(acts):
                wsd = spool.tile([128, ISP // 128, H], BF, tag="wsd")
                for io in range(ISP // 128):
                    nc.sync.dma_start(
                        wsd[:, io, :], wsd_ext[io * 128 : (io + 1) * 128, :]
                    )
                for tt in range(T // 128):
                    for hc in range(2):
                        ps = pd.tile([128, 512], F32, tag="pd")
                        for io in range(ISP // 128):
                            nc.tensor.matmul(
                                ps,
                                acts[:, io, tt * 128 : (tt + 1) * 128],
                                wsd[:, io, hc * 512 : (hc + 1) * 512],
                                start=(io == 0),
                                stop=(io == ISP // 128 - 1),
                            )
                        y = y_pool.tile([128, 512], F32, tag="y")
                        nc.vector.tensor_copy(y, ps)
                        nc.sync.dma_start(
                            out_ext[
                                2 * C + tt * 128 : 2 * C + (tt + 1) * 128,
                                hc * 512 : (hc + 1) * 512,
                            ],
                            y,
                        )

            gate_up(0)
            wdn0 = load_wdn(0)
            gate_up(1)
            down(0, wdn0)
            wdn1 = load_wdn(1)
            acts = shared_gate_up()
            down(1, wdn1)
            shared_down(acts)

    nc.compile()
    return nc


@functools.lru_cache(maxsize=4)
def _compiled(C: int):
    return _build_nc(C)


def _route(x, w_gate):
    """Mirror the reference router: softmax, top-4 (desc, ties -> lower
    index), renormalize."""
    logits = x @ w_gate  # f32 [T, E]
    m = logits.max(axis=-1, keepdims=True)
    p = np.exp(logits - m)
    p /= p.sum(axis=-1, keepdims=True)
    order = np.argsort(-p, axis=-1, kind="stable")[:, :K]  # [T, K]
    topw = np.take_along_axis(p, order, axis=-1)
    topw = topw / topw.sum(axis=-1, keepdims=True)
    return order, topw


def kernel(hidden_states, w_gate, w_moe_gate_up, w_moe_down,
           w_shared_gate_up, w_shared_down):
    from concourse.bass_utils import run_bass_kernel_spmd

    x = np.asarray(hidden_states, dtype=np.float32)
    w_gate = np.asarray(w_gate, dtype=np.float32)
    w_moe_gate_up = np.asarray(w_moe_gate_up, dtype=np.float32)
    w_moe_down = np.asarray(w_moe_down, dtype=np.float32)
    w_shared_gate_up = np.asarray(w_shared_gate_up, dtype=np.float32)
    w_shared_down = np.asarray(w_shared_down, dtype=np.float32)

    topk_ids, topk_w = _route(x, w_gate)

    # per-expert token lists + combine weights
    rows_e = []
    wts_e = []
    for e in range(E):
        r, k = np.nonzero(topk_ids == e)
        rows_e.append(r)
        wts_e.append(topk_w[r, k].astype(np.float32))
    counts = np.array([len(r) for r in rows_e])
    C = max(128, int(np.ceil(counts.max() / 64)) * 64)

    nc = _compiled(C)

    xT_bf = np.ascontiguousarray(x.T).astype(BF16)  # [H, T]
    w_gu_bf = w_moe_gate_up.astype(BF16)            # [E, H, 2I]
    w_dn_bf = w_moe_down.astype(BF16)               # [E, I, H]

    S = IS // N_CORES  # 352
    in_maps = []
    for c in range(N_CORES):
        xe = np.zeros((H, 2 * C), dtype=BF16)
        wtok = np.zeros((2 * C, 1), dtype=np.float32)
        for j, e in enumerate((2 * c, 2 * c + 1)):
            cnt = counts[e]
            xe[:, j * C : j * C + cnt] = xT_bf[:, rows_e[e]]
            wtok[j * C : j * C + cnt, 0] = wts_e[e]
        wsgu = np.zeros((H, 2 * ISP), dtype=BF16)
        wsgu[:, :S] = w_shared_gate_up[:, c * S : (c + 1) * S].astype(BF16)
        wsgu[:, ISP : ISP + S] = w_shared_gate_up[
            :, IS + c * S : IS + (c + 1) * S
        ].astype(BF16)
        wsd = np.zeros((ISP, H), dtype=BF16)
        wsd[:S] = w_shared_down[c * S : (c + 1) * S].astype(BF16)
        in_maps.append(
            {
                "xT": xT_bf,
                "xe": xe,
                "wtok": wtok,
                "w_gu": w_gu_bf[2 * c : 2 * c + 2],
                "w_dn": w_dn_bf[2 * c : 2 * c + 2],
                "w_sgu": wsgu,
                "w_sd": wsd,
            }
        )

    res = run_bass_kernel_spmd(nc, in_maps, core_ids=list(range(N_CORES)))

    out = np.zeros((T, H), dtype=np.float32)
    for c in range(N_CORES):
        r = res.results[c]["out"]
        for j, e in enumerate((2 * c, 2 * c + 1)):
            cnt = counts[e]
            out[rows_e[e]] += r[j * C : j * C + cnt]
        out += r[2 * C :]
    return out


# revision 11
# speedup vs baseline: 1.1276x; 1.1276x over previous
"""BailingMoe (T=1024, H=1024, E=16, K=4, I=1408, IS=2816) on 8 TRN2 cores.

Strategy: expert-parallel, 2 experts per core. The router (x @ w_gate,
softmax, top-4, renorm — 0.02% of FLOPs) runs on host as part of input
sharding: tokens are gathered per expert (capacity C = max expert load,
rounded up to 64) and shipped pre-transposed. Each core computes its two
experts' MLPs on their gathered tokens (bf16 matmuls, f32 PSUM
accumulation), scales rows by the renormalized combine weight, and also
computes a tensor-parallel slice (IS/8 = 352, zero-padded to 384) of the
shared expert over all tokens. Host unshards: scatter-add the weighted
expert outputs and sum the 8 shared-expert partials.
"""

import functools

import numpy as np
import ml_dtypes

T = 1024
H = 1024
E = 16
K = 4
I = 1408
IS = 2816
ISP = 384          # padded per-core shared-expert slice (2816/8 = 352 -> 384)
TI = I // 128      # 11 intermediate tiles per routed expert
N_CORES = 8

BF16 = ml_dtypes.bfloat16


def _build_nc(C: int):
    import concourse.bass as bass  # noqa: F401  (bacc needs bass loaded)
    import concourse.mybir as mybir
    import concourse.tile as tile
    from concourse import bacc

    BF = mybir.dt.bfloat16
    F32 = mybir.dt.float32

    nc = bacc.Bacc(None, target_bir_lowering=False, debug=False)

    # All bulk inputs are pre-tiled on host so every DMA is contiguous
    # (DMA packets cap at 2KB; strided 256B descriptors run ~8x slower).
    xT_ext = nc.declare_dram_parameter("xT", [128, 8, T], BF, isOutput=False)
    xe_ext = nc.declare_dram_parameter("xe", [128, 8, 2 * C], BF, isOutput=False)
    wtok_ext = nc.declare_dram_parameter("wtok", [2 * C, 1], F32, isOutput=False)
    wgu_ext = nc.declare_dram_parameter(
        "w_gu", [2, 2 * TI, 128, 8, 128], BF, isOutput=False
    )
    wdn_ext = nc.declare_dram_parameter("w_dn", [2, I, H], BF, isOutput=False)
    wsgu_ext = nc.declare_dram_parameter(
        "w_sgu", [2 * (ISP // 128), 128, 8, 128], BF, isOutput=False
    )
    wsd_ext = nc.declare_dram_parameter("w_sd", [ISP, H], BF, isOutput=False)
    out_ext = nc.declare_dram_parameter("out", [2 * C + T, H], F32, isOutput=True)

    # ragged token tiles per expert (C is a multiple of 64)
    tt_sizes = [128] * (C // 128) + ([C % 128] if C % 128 else [])

    with tile.TileContext(nc) as tc:
        with (
            tc.tile_pool(name="xpool", bufs=1) as xpool,
            tc.tile_pool(name="wg_pool", bufs=4) as wg_pool,
            tc.tile_pool(name="wu_pool", bufs=4) as wu_pool,
            tc.tile_pool(name="wdn_pool", bufs=2) as wdn_pool,
            tc.tile_pool(name="act_pool", bufs=2) as act_pool,
            tc.tile_pool(name="spool", bufs=1) as spool,
            tc.tile_pool(name="tmp_pool", bufs=3) as tmp_pool,
            tc.tile_pool(name="y_pool", bufs=4) as y_pool,
            tc.tile_pool(name="wt_pool", bufs=1) as wt_pool,
            tc.tile_pool(name="pg", bufs=2, space="PSUM") as pg,
            tc.tile_pool(name="pd", bufs=3, space="PSUM") as pd,
        ):
            # ---- persistent inputs ----
            xsb = xpool.tile([128, 8, T], BF, tag="xsb")
            nc.sync.dma_start(xsb[:], xT_ext[:])
            xesb = xpool.tile([128, 8, 2 * C], BF, tag="xesb")
            nc.sync.dma_start(xesb[:], xe_ext[:])

            # combine weights, one [tw, 1] strip per (expert, token tile)
            wt_tiles = {}
            for e in range(2):
                t0 = 0
                for tt, tw in enumerate(tt_sizes):
                    w = wt_pool.tile([128, 1], F32, tag=f"wt_{e}_{tt}")
                    nc.sync.dma_start(
                        w[:tw], wtok_ext[e * C + t0 : e * C + t0 + tw, :]
                    )
                    wt_tiles[(e, tt)] = w
                    t0 += tw

            acte = [None, None]

            def gate_up(e):
                """gu^T tiles [128 of 2I, C] -> silu(g)*u -> acte[e][128, TI, C]"""
                cb = e * C
                a = act_pool.tile([128, TI, C], BF, tag="acte")
                acte[e] = a
                for i in range(TI):
                    wg = wg_pool.tile([128, 8, 128], BF, tag="wg")
                    nc.sync.dma_start(wg[:], wgu_ext[e, i])
                    wu = wu_pool.tile([128, 8, 128], BF, tag="wu")
                    nc.sync.dma_start(wu[:], wgu_ext[e, TI + i])
                    psg = pg.tile([128, 512], F32, tag="psg", name="psg")[:, :C]
                    psu = pg.tile([128, 512], F32, tag="psu", name="psu")[:, :C]
                    for h in range(8):
                        rhs = xesb[:, h, cb : cb + C]
                        nc.tensor.matmul(
                            psg, wg[:, h, :], rhs, start=(h == 0), stop=(h == 7)
                        )
                        nc.tensor.matmul(
                            psu, wu[:, h, :], rhs, start=(h == 0), stop=(h == 7)
                        )
                    tmp = tmp_pool.tile([128, 512], F32, tag="tmp", name="tmp")[:, :C]
                    nc.scalar.activation(
                        tmp, psg, mybir.ActivationFunctionType.Silu
                    )
                    nc.vector.tensor_mul(a[:, i, :], tmp, psu)

            def load_wdn(e):
                w = wdn_pool.tile([128, TI, H], BF, tag="wdn")
                for io in range(TI):
                    nc.sync.dma_start(
                        w[:, io, :], wdn_ext[e, io * 128 : (io + 1) * 128, :]
                    )
                return w

            def down(e, wdn):
                a = acte[e]
                t0 = 0
                for tt, tw in enumerate(tt_sizes):
                    for hc in range(2):
                        ps = pd.tile([128, 512], F32, tag="pd", name="pd")[:tw]
                        for io in range(TI):
                            nc.tensor.matmul(
                                ps,
                                a[:, io, t0 : t0 + tw],
                                wdn[:, io, hc * 512 : (hc + 1) * 512],
                                start=(io == 0),
                                stop=(io == TI - 1),
                            )
                        y = y_pool.tile([128, 512], F32, tag="y", name="y")[:tw]
                        nc.vector.tensor_scalar_mul(
                            y, ps, wt_tiles[(e, tt)][:tw]
                        )
                        nc.sync.dma_start(
                            out_ext[
                                e * C + t0 : e * C + t0 + tw,
                                hc * 512 : (hc + 1) * 512,
                            ],
                            y,
                        )
                    t0 += tw

            def shared_gate_up():
                acts = spool.tile([128, ISP // 128, T], BF, tag="acts")
                for j in range(ISP // 128):
                    wsg = wg_pool.tile([128, 8, 128], BF, tag="wsg")
                    nc.sync.dma_start(wsg[:], wsgu_ext[j])
                    wsu = wu_pool.tile([128, 8, 128], BF, tag="wsu")
                    nc.sync.dma_start(wsu[:], wsgu_ext[ISP // 128 + j])
                    for tch in range(2):
                        psg = pg.tile([128, 512], F32, tag="psg")
                        psu = pg.tile([128, 512], F32, tag="psu")
                        rhs = xsb[:, :, tch * 512 : (tch + 1) * 512]
                        for h in range(8):
                            nc.tensor.matmul(
                                psg,
                                wsg[:, h, :],
                                rhs[:, h, :],
                                start=(h == 0),
                                stop=(h == 7),
                            )
                            nc.tensor.matmul(
                                psu,
                                wsu[:, h, :],
                                rhs[:, h, :],
                                start=(h == 0),
                                stop=(h == 7),
                            )
                        tmp = tmp_pool.tile([128, 512], F32, tag="tmp")
                        nc.scalar.activation(
                            tmp, psg, mybir.ActivationFunctionType.Silu
                        )
                        nc.vector.tensor_mul(
                            acts[:, j, tch * 512 : (tch + 1) * 512], tmp, psu
                        )
                return acts

            def shared_down(acts):
                wsd = spool.tile([128, ISP // 128, H], BF, tag="wsd")
                for io in range(ISP // 128):
                    nc.sync.dma_start(
                        wsd[:, io, :], wsd_ext[io * 128 : (io + 1) * 128, :]
                    )
                for tt in range(T // 128):
                    for hc in range(2):
                        ps = pd.tile([128, 512], F32, tag="pd")
                        for io in range(ISP // 128):
                            nc.tensor.matmul(
                                ps,
                                acts[:, io, tt * 128 : (tt + 1) * 128],
                                wsd[:, io, hc * 512 : (hc + 1) * 512],
                                start=(io == 0),
                                stop=(io == ISP // 128 - 1),
                            )
                        y = y_pool.tile([128, 512], F32, tag="y")
                        nc.vector.tensor_copy(y, ps)
                        nc.sync.dma_start(
                            out_ext[
                                2 * C + tt * 128 : 2 * C + (tt + 1) * 128,
                                hc * 512 : (hc + 1) * 512,
                            ],
                            y,
                        )

            gate_up(0)
            wdn0 = load_wdn(0)
            gate_up(1)
            down(0, wdn0)
            wdn1 = load_wdn(1)
            acts = shared_gate_up()
            down(1, wdn1)
            shared_down(acts)

    nc.compile()
    return nc


@functools.lru_cache(maxsize=4)
def _compiled(C: int):
    return _build_nc(C)


def _route(x, w_gate):
    """Mirror the reference router: softmax, top-4 (desc, ties -> lower
    index), renormalize."""
    logits = x @ w_gate  # f32 [T, E]
    m = logits.max(axis=-1, keepdims=True)
    p = np.exp(logits - m)
    p /= p.sum(axis=-1, keepdims=True)
    order = np.argsort(-p, axis=-1, kind="stable")[:, :K]  # [T, K]
    topw = np.take_along_axis(p, order, axis=-1)
    topw = topw / topw.sum(axis=-1, keepdims=True)
    return order, topw


def kernel(hidden_states, w_gate, w_moe_gate_up, w_moe_down,
           w_shared_gate_up, w_shared_down):
    from concourse.bass_utils import run_bass_kernel_spmd

    x = np.asarray(hidden_states, dtype=np.float32)
    w_gate = np.asarray(w_gate, dtype=np.float32)
    w_moe_gate_up = np.asarray(w_moe_gate_up, dtype=np.float32)
    w_moe_down = np.asarray(w_moe_down, dtype=np.float32)
    w_shared_gate_up = np.asarray(w_shared_gate_up, dtype=np.float32)
    w_shared_down = np.asarray(w_shared_down, dtype=np.float32)

    topk_ids, topk_w = _route(x, w_gate)

    # per-expert token lists + combine weights
    rows_e = []
    wts_e = []
    for e in range(E):
        r, k = np.nonzero(topk_ids == e)
        rows_e.append(r)
        wts_e.append(topk_w[r, k].astype(np.float32))
    counts = np.array([len(r) for r in rows_e])
    C = max(128, int(np.ceil(counts.max() / 64)) * 64)

    nc = _compiled(C)

    def tile_po(a):
        """[H=o*128+p, F] -> contiguous [128(p), 8(o), F]"""
        return np.ascontiguousarray(
            a.reshape(8, 128, a.shape[-1]).transpose(1, 0, 2)
        )

    xT_bf = np.ascontiguousarray(x.T).astype(BF16)  # [H, T]
    xT_t = tile_po(xT_bf)                           # [128, 8, T]
    # [E, H, 2I] -> [E, 22(col tile), 128(p), 8(o), 128(c)], contiguous
    w_gu_t = np.ascontiguousarray(
        w_moe_gate_up.astype(BF16)
        .reshape(E, 8, 128, 2 * TI, 128)
        .transpose(0, 3, 2, 1, 4)
    )
    w_dn_bf = w_moe_down.astype(BF16)               # [E, I, H]

    S = IS // N_CORES  # 352
    in_maps = []
    for c in range(N_CORES):
        xe = np.zeros((H, 2 * C), dtype=BF16)
        wtok = np.zeros((2 * C, 1), dtype=np.float32)
        for j, e in enumerate((2 * c, 2 * c + 1)):
            cnt = counts[e]
            xe[:, j * C : j * C + cnt] = xT_bf[:, rows_e[e]]
            wtok[j * C : j * C + cnt, 0] = wts_e[e]
        wsgu = np.zeros((H, 2 * ISP), dtype=BF16)
        wsgu[:, :S] = w_shared_gate_up[:, c * S : (c + 1) * S].astype(BF16)
        wsgu[:, ISP : ISP + S] = w_shared_gate_up[
            :, IS + c * S : IS + (c + 1) * S
        ].astype(BF16)
        # [H, 2*ISP] -> [6(col tile), 128(p), 8(o), 128(c)]
        wsgu_t = np.ascontiguousarray(
            wsgu.reshape(8, 128, 2 * (ISP // 128), 128).transpose(2, 1, 0, 3)
        )
        wsd = np.zeros((ISP, H), dtype=BF16)
        wsd[:S] = w_shared_down[c * S : (c + 1) * S].astype(BF16)
        in_maps.append(
            {
                "xT": xT_t,
                "xe": tile_po(xe),
                "wtok": wtok,
                "w_gu": w_gu_t[2 * c : 2 * c + 2],
                "w_dn": w_dn_bf[2 * c : 2 * c + 2],
                "w_sgu": wsgu_t,
                "w_sd": wsd,
            }
        )

    res = run_bass_kernel_spmd(nc, in_maps, core_ids=list(range(N_CORES)))

    out = np.zeros((T, H), dtype=np.float32)
    for c in range(N_CORES):
        r = res.results[c]["out"]
        for j, e in enumerate((2 * c, 2 * c + 1)):
            cnt = counts[e]
            out[rows_e[e]] += r[j * C : j * C + cnt]
        out += r[2 * C :]
    return out
